# revision 1
# baseline (speedup 1.0000x reference)
"""Trainium2 Bass kernel for nn_AttentionModule (Bahdanau-style attention).

Reference computation (S=512, B=64, H=1024, F=2H):
    cat    = concat([hidden bcast to (S,B,H), encoder_states], -1)      [S,B,2H]
    scores = tanh(cat @ W_attn.T + b_attn) @ W_attn2.T + b_attn2        [S,B,1]
    attn   = softmax(scores[..., 0].T, axis=-1)                         [B,S]
    applied= einsum("bs,sbh->bh", attn, encoder_states)                 [B,H]
    out    = tanh(concat([decoder_out, applied], -1) @ W_comb.T + b_comb)

Sharding: data-parallel over B across 8 cores (8 batch rows per core).
All heavy math stays on-device; the host only slices, transposes and casts
the per-core shards.

Per-core structure:
  - enc_t [8, 1024, 512] bf16: encoder slice with H on partitions. One 2MB
    DMA per batch row (3D access pattern).
  - Main matmul per (b, ft): T^T[f, s] = sum_h W2T[h, f] * encT[h, s] with
    the weight chunk stationary, bf16 at full PE rate, fp32 PSUM.
  - tanh fused on ScalarE with per-partition bias b_attn[f] + hid_part[b, f]
    (hid_part computed on device in a preamble).
  - scores via PE matmul contracting f: lhsT = W_attn2 replicated to 8 cols
    (all psum rows identical -> row b used directly, no partition shifts).
  - softmax over s on 8 partitions (reduce_max(negate) -> Exp with bias and
    fused accumulate -> reciprocal -> scale).
  - attention row broadcast across partitions via a DRAM bounce DMA.
  - applied^T[h, b] on VectorE: multiply resident encT tiles by the broadcast
    attention row, reduce along s. Written column-wise into appT (fp32 output)
    and converted per-b to bf16 for the final matmul.
  - Final combine matmul (bf16) with biases folded as K=1 matmul terms.

Known pitfalls baked into this implementation:
  - bf16 input arrays with tiny rows (16B) get corrupted on the host->device
    path, so every small tensor ships as fp32 and is cast on device.
  - fp32 matmuls run at 1/4 rate; fp32r is full rate but only PE/DMA may
    touch f32r-typed tensors; bf16 everywhere is simplest at full rate.
  - 16/32-bit matmul operand mixing is rejected by the compiler.
  - Multi-dim rearrange DMAs with 16-byte inner blocks corrupt data on HW;
    only used with >=1KB inner blocks here (the encoder load).
"""

import numpy as np

S, B, H = 512, 64, 1024
F = 2 * H
NCORES = 8
BL = B // NCORES          # 8 batch rows per core
KH = H // 128             # 8 contraction chunks over H
KF = F // 128             # 16 feature tiles

_CACHE = {}


def _build(num_devices=NCORES):
    from contextlib import ExitStack

    import concourse.tile as tile
    from concourse import bacc, mybir
    from concourse.masks import make_identity

    f32 = mybir.dt.float32
    bf16 = mybir.dt.bfloat16
    AF = mybir.ActivationFunctionType
    ALU = mybir.AluOpType
    AX = mybir.AxisListType

    nc = bacc.Bacc("TRN2", target_bir_lowering=False, debug=False,
                   num_devices=num_devices)

    enc_t = nc.dram_tensor("enc_t", [BL, H, S], bf16, kind="ExternalInput").ap()
    wat = nc.dram_tensor("wat", [F, F], bf16, kind="ExternalInput").ap()
    wct = nc.dram_tensor("wct", [F, H], bf16, kind="ExternalInput").ap()
    hidT = nc.dram_tensor("hidT", [H, BL], f32, kind="ExternalInput").ap()
    decT = nc.dram_tensor("decT", [H, BL], f32, kind="ExternalInput").ap()
    w2rep = nc.dram_tensor("w2rep", [F, BL], f32, kind="ExternalInput").ap()
    b_attn_d = nc.dram_tensor("b_attn", [1, F], f32, kind="ExternalInput").ap()
    b_comb_d = nc.dram_tensor("b_comb", [1, H], f32, kind="ExternalInput").ap()
    out_d = nc.dram_tensor("out", [BL, H], f32, kind="ExternalOutput").ap()
    appT_d = nc.dram_tensor("appliedT", [H, BL], f32,
                            kind="ExternalOutput").ap()

    with tile.TileContext(nc) as tc:
        with ExitStack() as ctx:
            consts = ctx.enter_context(tc.tile_pool(name="consts", bufs=1))
            enct_pool = ctx.enter_context(tc.tile_pool(name="enct", bufs=2))
            w1_pool = ctx.enter_context(tc.tile_pool(name="w1t", bufs=2))
            tanh_pool = ctx.enter_context(tc.tile_pool(name="tanh", bufs=18))
            attn_pool = ctx.enter_context(tc.tile_pool(name="attn", bufs=2))
            abc_pool = ctx.enter_context(tc.tile_pool(name="abc", bufs=2))
            dram_pool = ctx.enter_context(
                tc.tile_pool(name="dram", bufs=2, space="DRAM"))
            scr_pool = ctx.enter_context(tc.tile_pool(name="scr", bufs=2))
            small_pool = ctx.enter_context(tc.tile_pool(name="small", bufs=4))
            wct_pool = ctx.enter_context(tc.tile_pool(name="wct", bufs=4))
            psT_pool = ctx.enter_context(
                tc.tile_pool(name="psT", bufs=2, space="PSUM"))
            psSc_pool = ctx.enter_context(
                tc.tile_pool(name="psSc", bufs=2, space="PSUM"))
            psPre_pool = ctx.enter_context(
                tc.tile_pool(name="psPre", bufs=2, space="PSUM"))
            psOut_pool = ctx.enter_context(
                tc.tile_pool(name="psOut", bufs=2, space="PSUM"))

            # ---- encoder prefetch for b=0 (emitted first so its DMA leads) --
            def load_enct(b):
                t = enct_pool.tile([128, KH * S], bf16, tag="enct",
                                   name="enct")
                nc.sync.dma_start(
                    t.rearrange("p (k s) -> p k s", s=S),
                    enc_t[b].rearrange("(k p) s -> p k s", p=128))
                return t

            enct_tiles = {0: load_enct(0)}

            # ---- W2T chunk 0 early so the first main matmul can start ----
            w2t_sb = consts.tile([128, KH * F], bf16)
            nc.sync.dma_start(w2t_sb[:, 0:F], wat[H:H + 128, :])

            # ---- small constants (shipped fp32, cast on device) ----
            identity = consts.tile([128, 128], f32)
            make_identity(nc, identity[:])
            ones_bf = consts.tile([1, BL], bf16)
            nc.vector.memset(ones_bf[:], 1.0)
            b_attn_32 = consts.tile([1, F], f32)
            nc.sync.dma_start(b_attn_32[:], b_attn_d[:])
            b_attn_sb = consts.tile([1, F], bf16)
            nc.vector.tensor_copy(b_attn_sb[:], b_attn_32[:])
            b_comb_32 = consts.tile([1, H], f32)
            nc.sync.dma_start(b_comb_32[:], b_comb_d[:])
            b_comb_sb = consts.tile([1, H], bf16)
            nc.vector.tensor_copy(b_comb_sb[:], b_comb_32[:])

            hidT_32 = consts.tile([128, KH * BL], f32)
            decT_32 = consts.tile([128, KH * BL], f32)
            w2rep_32 = consts.tile([128, KF * BL], f32)
            for kc in range(KH):
                nc.sync.dma_start(hidT_32[:, kc * BL:(kc + 1) * BL],
                                  hidT[kc * 128:(kc + 1) * 128, :])
                nc.sync.dma_start(decT_32[:, kc * BL:(kc + 1) * BL],
                                  decT[kc * 128:(kc + 1) * 128, :])
            for ft in range(KF):
                nc.sync.dma_start(w2rep_32[:, ft * BL:(ft + 1) * BL],
                                  w2rep[ft * 128:(ft + 1) * 128, :])
            hidT_sb = consts.tile([128, KH * BL], bf16)
            nc.vector.tensor_copy(hidT_sb[:], hidT_32[:])
            decT_sb = consts.tile([128, KH * BL], bf16)
            nc.vector.tensor_copy(decT_sb[:], decT_32[:])
            w2rep_sb = consts.tile([128, KF * BL], bf16)
            nc.vector.tensor_copy(w2rep_sb[:], w2rep_32[:])

            hidbT_sb = consts.tile([128, KF * BL], f32)
            appT_sb = consts.tile([128, KH * BL], f32)
            appT_bf = consts.tile([128, KH * BL], bf16)

            # ---- hid_part preamble: hidb[b, f] = hidden @ W1.T + b_attn ----
            hidb_row = consts.tile([BL, F], f32)
            for fc in range(F // 512):
                ph = psPre_pool.tile([BL, 512], f32, tag="pre", name=f"ph{fc}")
                for kc in range(KH):
                    w1c = w1_pool.tile([128, 512], bf16, tag="w1t", name="w1c")
                    nc.sync.dma_start(
                        w1c[:], wat[kc * 128:(kc + 1) * 128,
                                    fc * 512:(fc + 1) * 512])
                    nc.tensor.matmul(
                        ph[:], hidT_sb[:, kc * BL:(kc + 1) * BL], w1c[:],
                        start=(kc == 0), stop=False)
                nc.tensor.matmul(
                    ph[:], ones_bf[:], b_attn_sb[:, fc * 512:(fc + 1) * 512],
                    start=False, stop=True)
                nc.vector.tensor_copy(hidb_row[:, fc * 512:(fc + 1) * 512],
                                      ph[:])
            # transpose [8, 2048] -> hidbT_sb [128, KF*8] (f on partitions)
            for ft in range(KF):
                ptp = psPre_pool.tile([128, BL], f32, tag="pre", name="ptp")
                nc.tensor.transpose(ptp[:],
                                    hidb_row[:, ft * 128:(ft + 1) * 128],
                                    identity[:BL, :BL])
                nc.vector.tensor_copy(hidbT_sb[:, ft * BL:(ft + 1) * BL],
                                      ptp[:])

            # ---- remaining W2T chunks ----
            for kc in range(1, KH):
                nc.sync.dma_start(
                    w2t_sb[:, kc * F:(kc + 1) * F],
                    wat[H + kc * 128: H + (kc + 1) * 128, :])

            # ---- main loop over local batch rows ----
            for b in range(BL):
                if b + 1 < BL:
                    enct_tiles[b + 1] = load_enct(b + 1)
                et = enct_tiles.pop(b)

                def etk(kc):
                    return et[:, kc * S:(kc + 1) * S]

                th = []
                for ft in range(KF):
                    pT = psT_pool.tile([128, S], f32, tag="pT", name="pT")
                    for kc in range(KH):
                        nc.tensor.matmul(
                            pT[:],
                            w2t_sb[:, kc * F + ft * 128:
                                   kc * F + (ft + 1) * 128],
                            etk(kc),
                            start=(kc == 0), stop=(kc == KH - 1))
                    t = tanh_pool.tile([128, S], bf16, tag="tanh", name="tanh")
                    nc.scalar.activation(
                        t[:], pT[:], AF.Tanh,
                        bias=hidbT_sb[:, ft * BL + b: ft * BL + b + 1],
                        scale=1.0)
                    th.append(t)

                psc = psSc_pool.tile([BL, S], f32, tag="psc", name="psc")
                for ft in range(KF):
                    nc.tensor.matmul(
                        psc[:],
                        w2rep_sb[:, ft * BL:(ft + 1) * BL],
                        th[ft][:],
                        start=(ft == 0), stop=(ft == KF - 1))

                negmax = small_pool.tile([BL, 1], f32, tag="negmax",
                                         name="negmax")
                nc.vector.reduce_max(negmax[:], psc[:], axis=AX.X, negate=True)
                attn = attn_pool.tile([BL, S], bf16, tag="attn", name="attn")
                sumexp = small_pool.tile([BL, 1], f32, tag="sumexp",
                                         name="sumexp")
                nc.scalar.activation(attn[:], psc[:], AF.Exp,
                                     bias=negmax[:], scale=1.0,
                                     accum_out=sumexp[:])
                recip = small_pool.tile([BL, 1], f32, tag="recip", name="recip")
                nc.vector.reciprocal(recip[:], sumexp[:])
                nc.vector.tensor_scalar_mul(attn[:], attn[:], recip[:])

                # broadcast attn row across 128 partitions via DRAM bounce
                attn_dr = dram_pool.tile([1, S], bf16, tag="attn_dr",
                                         name="attn_dr")
                nc.sync.dma_start(attn_dr[:], attn[0:1, :])
                abc = abc_pool.tile([128, S], bf16, tag="abc", name="abc")
                nc.sync.dma_start(abc[:],
                                  attn_dr[0:1, :].to_broadcast((128, S)))

                for kc in range(KH):
                    scr = scr_pool.tile([128, S], f32, tag="scr", name="scr")
                    nc.vector.tensor_tensor(out=scr[:], in0=etk(kc),
                                            in1=abc[:], op=ALU.mult)
                    nc.vector.reduce_sum(
                        appT_sb[:, kc * BL + b: kc * BL + b + 1],
                        scr[:], axis=AX.X)
                nc.vector.tensor_copy(
                    appT_bf.rearrange("p (k b) -> p k b", b=BL)[:, :, b],
                    appT_sb.rearrange("p (k b) -> p k b", b=BL)[:, :, b])

            # ---- final combine: out = tanh([dec | applied] @ Wc.T + b_comb) --
            pouts = [psOut_pool.tile([BL, 512], f32, tag="pout", name=f"po{i}")
                     for i in range(2)]
            for kc in range(2 * KH):
                if kc < KH:
                    lhs = decT_sb[:, kc * BL:(kc + 1) * BL]
                else:
                    lhs = appT_bf[:, (kc - KH) * BL:(kc - KH + 1) * BL]
                w = wct_pool.tile([128, H], bf16, tag="wct", name="wctt")
                nc.sync.dma_start(w[:], wct[kc * 128:(kc + 1) * 128, :])
                for fc in range(2):
                    nc.tensor.matmul(
                        pouts[fc][:], lhs, w[:, fc * 512:(fc + 1) * 512],
                        start=(kc == 0), stop=False)
            for fc in range(2):
                nc.tensor.matmul(
                    pouts[fc][:], ones_bf[:],
                    b_comb_sb[:, fc * 512:(fc + 1) * 512],
                    start=False, stop=True)

            out_sb = consts.tile([BL, H], f32)
            for fc in range(2):
                nc.scalar.activation(out_sb[:, fc * 512:(fc + 1) * 512],
                                     pouts[fc][:], AF.Tanh)
            nc.sync.dma_start(out_d[:], out_sb[:])
            for kc in range(KH):
                nc.sync.dma_start(appT_d[kc * 128:(kc + 1) * 128, :],
                                  appT_sb[:, kc * BL:(kc + 1) * BL])

    nc.compile()
    return nc


def _get_nc():
    if "nc" not in _CACHE:
        _CACHE["nc"] = _build()
    return _CACHE["nc"]


def make_in_maps(inputs):
    import ml_dtypes
    bf = ml_dtypes.bfloat16

    inp = {k: np.asarray(v, dtype=np.float32) for k, v in inputs.items()}
    hidden = inp["hidden"]
    decoder_out = inp["decoder_out"]
    encoder_states = inp["encoder_states"]
    W_attn = inp["W_attn"]
    b_attn = inp["b_attn"]
    W_attn2 = inp["W_attn2"]
    W_comb = inp["W_comb"]
    b_comb = inp["b_comb"]
    # b_attn2 shifts every score equally -> softmax-invariant, unused.

    wat = np.ascontiguousarray(W_attn.T).astype(bf)
    wct = np.ascontiguousarray(W_comb.T).astype(bf)
    w2rep = np.ascontiguousarray(np.repeat(W_attn2.reshape(F, 1), BL, axis=1))
    b_attn_2d = np.ascontiguousarray(b_attn.reshape(1, F))
    b_comb_2d = np.ascontiguousarray(b_comb.reshape(1, H))

    in_maps = []
    for c in range(NCORES):
        sl = slice(c * BL, (c + 1) * BL)
        in_maps.append({
            "enc_t": np.ascontiguousarray(
                encoder_states[:, sl, :].transpose(1, 2, 0)).astype(bf),
            "wat": wat,
            "wct": wct,
            "hidT": np.ascontiguousarray(hidden[sl].T),
            "decT": np.ascontiguousarray(decoder_out[sl].T),
            "w2rep": w2rep,
            "b_attn": b_attn_2d,
            "b_comb": b_comb_2d,
        })
    return in_maps


def kernel(**inputs):
    from concourse.bass_utils import run_bass_kernel_spmd

    in_maps = make_in_maps(inputs)
    nc = _get_nc()
    res = run_bass_kernel_spmd(nc, in_maps, list(range(NCORES)))
    out = np.concatenate([res.results[c]["out"] for c in range(NCORES)], axis=0)
    applied = np.concatenate(
        [np.ascontiguousarray(res.results[c]["appliedT"].T)
         for c in range(NCORES)], axis=0)
    return out.astype(np.float32), applied.astype(np.float32)



# revision 5
# speedup vs baseline: 1.2912x; 1.2912x over previous
"""Trainium2 Bass kernel for nn_AttentionModule (Bahdanau-style attention).

Reference computation (S=512, B=64, H=1024, F=2H):
    cat    = concat([hidden bcast to (S,B,H), encoder_states], -1)      [S,B,2H]
    scores = tanh(cat @ W_attn.T + b_attn) @ W_attn2.T + b_attn2        [S,B,1]
    attn   = softmax(scores[..., 0].T, axis=-1)                         [B,S]
    applied= einsum("bs,sbh->bh", attn, encoder_states)                 [B,H]
    out    = tanh(concat([decoder_out, applied], -1) @ W_comb.T + b_comb)

Sharding: data-parallel over B across 8 cores (8 batch rows per core).

v2 changes over the bf16 baseline (376.7us):
  - Main matmul (enc @ W_attn[:,H:].T, 17.2 GFLOP/core, ~215us of PE at
    bf16) runs in fp8e4 with perf_mode=DoubleRow: lhsT [128,2,128] and
    rhs [128,2,512] 3D APs pair adjacent K-chunks, contracting 256 per
    instruction.  W_attn is pre-scaled by 8192 on the host so all its
    values sit in fp8e4's normal range; the tanh activation un-scales
    via scale=1/8192.  Encoder ships twice: fp8 for the matmul, bf16
    for the applied-einsum (output precision).
  - Both encoder copies are pre-permuted on the host to [BL, 128, KH*S]
    so the SBUF load is a plain 2D contiguous DMA per batch row.
  - applied einsum: one fused tensor_tensor_reduce per chunk instead of
    tensor_tensor + reduce (halves DVE ops); 3 of 8 chunks offloaded to
    the idle GpSimd engine.
  - Scores (psc), softmax, preamble and final combine unchanged (bf16).

Known pitfalls baked into this implementation:
  - bf16 input arrays with tiny rows (16B) get corrupted on the
    host->device path, so every small tensor ships as fp32 and is cast
    on device.
  - fp8 without DoubleRow runs at bf16 speed; DoubleRow requires both
    operands fp8e4/e5 with the paired-K 3D AP layout.
  - 16/32-bit matmul operand mixing is rejected by the compiler.
  - mybir float8e4 == ml_dtypes.float8_e4m3 (IEEE, max normal 240),
    NOT float8_e4m3fn.
"""

import numpy as np

S, B, H = 512, 64, 1024
F = 2 * H
NCORES = 8
BL = B // NCORES          # 8 batch rows per core
KH = H // 128             # 8 contraction chunks over H
KF = F // 128             # 16 feature tiles
WSCALE = 8192.0           # host pre-scale on W_attn enc-half (fp8 range)

_CACHE = {}


def _build(num_devices=NCORES):
    from contextlib import ExitStack

    import concourse.tile as tile
    from concourse import bacc, mybir
    from concourse.masks import make_identity

    f32 = mybir.dt.float32
    bf16 = mybir.dt.bfloat16
    fp8 = mybir.dt.float8e4
    AF = mybir.ActivationFunctionType
    ALU = mybir.AluOpType
    AX = mybir.AxisListType
    DR = mybir.MatmulPerfMode.DoubleRow

    nc = bacc.Bacc("TRN2", target_bir_lowering=False, debug=False,
                   num_devices=num_devices)

    enc8_d = nc.dram_tensor("enc8", [BL, 128, KH * S], fp8,
                            kind="ExternalInput").ap()
    encb_d = nc.dram_tensor("encb", [BL, 128, KH * S], bf16,
                            kind="ExternalInput").ap()
    wat8 = nc.dram_tensor("wat8", [H, F], fp8, kind="ExternalInput").ap()
    w1t = nc.dram_tensor("w1t", [H, F], bf16, kind="ExternalInput").ap()
    wct = nc.dram_tensor("wct", [F, H], bf16, kind="ExternalInput").ap()
    hidT = nc.dram_tensor("hidT", [H, BL], f32, kind="ExternalInput").ap()
    decT = nc.dram_tensor("decT", [H, BL], f32, kind="ExternalInput").ap()
    w2rep = nc.dram_tensor("w2rep", [F, BL], f32, kind="ExternalInput").ap()
    b_attn_d = nc.dram_tensor("b_attn", [1, F], f32, kind="ExternalInput").ap()
    b_comb_d = nc.dram_tensor("b_comb", [1, H], f32, kind="ExternalInput").ap()
    out_d = nc.dram_tensor("out", [BL, H], f32, kind="ExternalOutput").ap()
    appT_d = nc.dram_tensor("appliedT", [H, BL], f32,
                            kind="ExternalOutput").ap()

    with tile.TileContext(nc) as tc:
        with ExitStack() as ctx:
            consts = ctx.enter_context(tc.tile_pool(name="consts", bufs=1))
            enc8_pool = ctx.enter_context(tc.tile_pool(name="enc8", bufs=2))
            encb_pool = ctx.enter_context(tc.tile_pool(name="encb", bufs=2))
            w1_pool = ctx.enter_context(tc.tile_pool(name="w1t", bufs=2))
            tanh_pool = ctx.enter_context(tc.tile_pool(name="tanh", bufs=18))
            attn_pool = ctx.enter_context(tc.tile_pool(name="attn", bufs=2))
            abc_pool = ctx.enter_context(tc.tile_pool(name="abc", bufs=2))
            dram_pool = ctx.enter_context(
                tc.tile_pool(name="dram", bufs=2, space="DRAM"))
            scr_pool = ctx.enter_context(tc.tile_pool(name="scr", bufs=2))
            scrg_pool = ctx.enter_context(tc.tile_pool(name="scrg", bufs=2))
            small_pool = ctx.enter_context(tc.tile_pool(name="small", bufs=4))
            wct_pool = ctx.enter_context(tc.tile_pool(name="wct", bufs=4))
            psT_pool = ctx.enter_context(
                tc.tile_pool(name="psT", bufs=2, space="PSUM"))
            psSc_pool = ctx.enter_context(
                tc.tile_pool(name="psSc", bufs=2, space="PSUM"))
            psPre_pool = ctx.enter_context(
                tc.tile_pool(name="psPre", bufs=2, space="PSUM"))
            psOut_pool = ctx.enter_context(
                tc.tile_pool(name="psOut", bufs=2, space="PSUM"))

            # ---- encoder prefetch (fp8 for matmul, bf16 for applied) ----
            def load_enc8(b):
                t = enc8_pool.tile([128, KH * S], fp8, tag="enc8", name="enc8")
                nc.sync.dma_start(t[:], enc8_d[b])
                return t

            def load_encb(b):
                t = encb_pool.tile([128, KH * S], bf16, tag="encb",
                                   name="encb")
                nc.sync.dma_start(t[:], encb_d[b])
                return t

            enc8_tiles = {0: load_enc8(0)}

            # ---- fp8 W2T chunks (2MB total) ----
            w2t_sb = consts.tile([128, KH * F], fp8)
            for kc in range(KH):
                nc.sync.dma_start(
                    w2t_sb[:, kc * F:(kc + 1) * F],
                    wat8[kc * 128:(kc + 1) * 128, :])
            w2t3 = w2t_sb.rearrange("p (k f) -> p k f", f=F)

            encb_tiles = {0: load_encb(0)}

            # ---- small constants (shipped fp32, cast on device) ----
            identity = consts.tile([128, 128], f32)
            make_identity(nc, identity[:])
            ones_bf = consts.tile([1, BL], bf16)
            nc.vector.memset(ones_bf[:], 1.0)
            b_attn_32 = consts.tile([1, F], f32)
            nc.sync.dma_start(b_attn_32[:], b_attn_d[:])
            b_attn_sb = consts.tile([1, F], bf16)
            nc.vector.tensor_copy(b_attn_sb[:], b_attn_32[:])
            b_comb_32 = consts.tile([1, H], f32)
            nc.sync.dma_start(b_comb_32[:], b_comb_d[:])
            b_comb_sb = consts.tile([1, H], bf16)
            nc.vector.tensor_copy(b_comb_sb[:], b_comb_32[:])

            hidT_32 = consts.tile([128, KH * BL], f32)
            decT_32 = consts.tile([128, KH * BL], f32)
            w2rep_32 = consts.tile([128, KF * BL], f32)
            for kc in range(KH):
                nc.sync.dma_start(hidT_32[:, kc * BL:(kc + 1) * BL],
                                  hidT[kc * 128:(kc + 1) * 128, :])
                nc.sync.dma_start(decT_32[:, kc * BL:(kc + 1) * BL],
                                  decT[kc * 128:(kc + 1) * 128, :])
            for ft in range(KF):
                nc.sync.dma_start(w2rep_32[:, ft * BL:(ft + 1) * BL],
                                  w2rep[ft * 128:(ft + 1) * 128, :])
            hidT_sb = consts.tile([128, KH * BL], bf16)
            nc.vector.tensor_copy(hidT_sb[:], hidT_32[:])
            decT_sb = consts.tile([128, KH * BL], bf16)
            nc.vector.tensor_copy(decT_sb[:], decT_32[:])
            w2rep_sb = consts.tile([128, KF * BL], bf16)
            nc.vector.tensor_copy(w2rep_sb[:], w2rep_32[:])

            hidbT_sb = consts.tile([128, KF * BL], f32)
            appT_sb = consts.tile([128, KH * BL], f32)
            appT_bf = consts.tile([128, KH * BL], bf16)

            # ---- hid_part preamble: hidb[b, f] = hidden @ W1.T + b_attn ----
            hidb_row = consts.tile([BL, F], f32)
            for fc in range(F // 512):
                ph = psPre_pool.tile([BL, 512], f32, tag="pre", name=f"ph{fc}")
                for kc in range(KH):
                    w1c = w1_pool.tile([128, 512], bf16, tag="w1t", name="w1c")
                    nc.sync.dma_start(
                        w1c[:], w1t[kc * 128:(kc + 1) * 128,
                                    fc * 512:(fc + 1) * 512])
                    nc.tensor.matmul(
                        ph[:], hidT_sb[:, kc * BL:(kc + 1) * BL], w1c[:],
                        start=(kc == 0), stop=False)
                nc.tensor.matmul(
                    ph[:], ones_bf[:], b_attn_sb[:, fc * 512:(fc + 1) * 512],
                    start=False, stop=True)
                nc.vector.tensor_copy(hidb_row[:, fc * 512:(fc + 1) * 512],
                                      ph[:])
            # transpose [8, 2048] -> hidbT_sb [128, KF*8] (f on partitions)
            for ft in range(KF):
                ptp = psPre_pool.tile([128, BL], f32, tag="pre", name="ptp")
                nc.tensor.transpose(ptp[:],
                                    hidb_row[:, ft * 128:(ft + 1) * 128],
                                    identity[:BL, :BL])
                nc.vector.tensor_copy(hidbT_sb[:, ft * BL:(ft + 1) * BL],
                                      ptp[:])

            # ---- main loop over local batch rows ----
            for b in range(BL):
                if b + 1 < BL:
                    enc8_tiles[b + 1] = load_enc8(b + 1)
                    encb_tiles[b + 1] = load_encb(b + 1)
                e8 = enc8_tiles.pop(b)
                eb = encb_tiles.pop(b)
                e83 = e8.rearrange("p (k s) -> p k s", s=S)

                def ebk(kc):
                    return eb[:, kc * S:(kc + 1) * S]

                th = []
                for ft in range(KF):
                    pT = psT_pool.tile([128, S], f32, tag="pT", name="pT")
                    for kc2 in range(KH // 2):
                        nc.tensor.matmul(
                            pT[:],
                            w2t3[:, 2 * kc2:2 * kc2 + 2,
                                 ft * 128:(ft + 1) * 128],
                            e83[:, 2 * kc2:2 * kc2 + 2, :],
                            start=(kc2 == 0), stop=(kc2 == KH // 2 - 1),
                            perf_mode=DR)
                    t = tanh_pool.tile([128, S], bf16, tag="tanh", name="tanh")
                    nc.scalar.activation(
                        t[:], pT[:], AF.Tanh,
                        bias=hidbT_sb[:, ft * BL + b: ft * BL + b + 1],
                        scale=1.0 / WSCALE)
                    th.append(t)

                psc = psSc_pool.tile([BL, S], f32, tag="psc", name="psc")
                for ft in range(KF):
                    nc.tensor.matmul(
                        psc[:],
                        w2rep_sb[:, ft * BL:(ft + 1) * BL],
                        th[ft][:],
                        start=(ft == 0), stop=(ft == KF - 1))

                negmax = small_pool.tile([BL, 1], f32, tag="negmax",
                                         name="negmax")
                nc.vector.reduce_max(negmax[:], psc[:], axis=AX.X, negate=True)
                attn = attn_pool.tile([BL, S], bf16, tag="attn", name="attn")
                sumexp = small_pool.tile([BL, 1], f32, tag="sumexp",
                                         name="sumexp")
                nc.scalar.activation(attn[:], psc[:], AF.Exp,
                                     bias=negmax[:], scale=1.0,
                                     accum_out=sumexp[:])
                recip = small_pool.tile([BL, 1], f32, tag="recip", name="recip")
                nc.vector.reciprocal(recip[:], sumexp[:])
                nc.vector.tensor_scalar_mul(attn[:], attn[:], recip[:])

                # broadcast attn row across 128 partitions via DRAM bounce
                attn_dr = dram_pool.tile([1, S], bf16, tag="attn_dr",
                                         name="attn_dr")
                nc.sync.dma_start(attn_dr[:], attn[0:1, :])
                abc = abc_pool.tile([128, S], bf16, tag="abc", name="abc")
                nc.sync.dma_start(abc[:],
                                  attn_dr[0:1, :].to_broadcast((128, S)))

                # applied^T column b: fused mult+accum per chunk on VectorE
                # (tensor_tensor_reduce hard-faults the DVE on HW; STT with
                # accum_out is the working single-op mult+free-axis-sum)
                for kc in range(KH):
                    col = appT_sb[:, kc * BL + b: kc * BL + b + 1]
                    scr = scr_pool.tile([128, S], bf16, tag="scr",
                                        name="scr")
                    nc.vector.scalar_tensor_tensor(
                        out=scr[:], in0=ebk(kc), scalar=1.0, in1=abc[:],
                        op0=ALU.mult, op1=ALU.mult, accum_out=col)
                nc.vector.tensor_copy(
                    appT_bf.rearrange("p (k b) -> p k b", b=BL)[:, :, b],
                    appT_sb.rearrange("p (k b) -> p k b", b=BL)[:, :, b])

            # ---- final combine: out = tanh([dec | applied] @ Wc.T + b_comb) --
            pouts = [psOut_pool.tile([BL, 512], f32, tag="pout", name=f"po{i}")
                     for i in range(2)]
            for kc in range(2 * KH):
                if kc < KH:
                    lhs = decT_sb[:, kc * BL:(kc + 1) * BL]
                else:
                    lhs = appT_bf[:, (kc - KH) * BL:(kc - KH + 1) * BL]
                w = wct_pool.tile([128, H], bf16, tag="wct", name="wctt")
                nc.sync.dma_start(w[:], wct[kc * 128:(kc + 1) * 128, :])
                for fc in range(2):
                    nc.tensor.matmul(
                        pouts[fc][:], lhs, w[:, fc * 512:(fc + 1) * 512],
                        start=(kc == 0), stop=False)
            for fc in range(2):
                nc.tensor.matmul(
                    pouts[fc][:], ones_bf[:],
                    b_comb_sb[:, fc * 512:(fc + 1) * 512],
                    start=False, stop=True)

            out_sb = consts.tile([BL, H], f32)
            for fc in range(2):
                nc.scalar.activation(out_sb[:, fc * 512:(fc + 1) * 512],
                                     pouts[fc][:], AF.Tanh)
            nc.sync.dma_start(out_d[:], out_sb[:])
            for kc in range(KH):
                nc.sync.dma_start(appT_d[kc * 128:(kc + 1) * 128, :],
                                  appT_sb[:, kc * BL:(kc + 1) * BL])

    nc.compile()
    return nc


def _get_nc():
    if "nc" not in _CACHE:
        _CACHE["nc"] = _build()
    return _CACHE["nc"]


def make_in_maps(inputs):
    import ml_dtypes
    bf = ml_dtypes.bfloat16
    f8 = ml_dtypes.float8_e4m3

    inp = {k: np.asarray(v, dtype=np.float32) for k, v in inputs.items()}
    hidden = inp["hidden"]
    decoder_out = inp["decoder_out"]
    encoder_states = inp["encoder_states"]
    W_attn = inp["W_attn"]
    b_attn = inp["b_attn"]
    W_attn2 = inp["W_attn2"]
    W_comb = inp["W_comb"]
    b_comb = inp["b_comb"]
    # b_attn2 shifts every score equally -> softmax-invariant, unused.

    watT = np.ascontiguousarray(W_attn.T)            # [2H, F]
    wat8 = np.ascontiguousarray(watT[H:] * WSCALE).astype(f8)
    w1t_np = np.ascontiguousarray(watT[:H]).astype(bf)
    wct = np.ascontiguousarray(W_comb.T).astype(bf)
    w2rep = np.ascontiguousarray(np.repeat(W_attn2.reshape(F, 1), BL, axis=1))
    b_attn_2d = np.ascontiguousarray(b_attn.reshape(1, F))
    b_comb_2d = np.ascontiguousarray(b_comb.reshape(1, H))

    in_maps = []
    for c in range(NCORES):
        sl = slice(c * BL, (c + 1) * BL)
        # [S, BL, H] -> [BL, H, S] -> [BL, 128, KH, S] (partition-major)
        encT = encoder_states[:, sl, :].transpose(1, 2, 0)
        encP = np.ascontiguousarray(
            encT.reshape(BL, KH, 128, S).transpose(0, 2, 1, 3)
        ).reshape(BL, 128, KH * S)
        in_maps.append({
            "enc8": encP.astype(f8),
            "encb": encP.astype(bf),
            "wat8": wat8,
            "w1t": w1t_np,
            "wct": wct,
            "hidT": np.ascontiguousarray(hidden[sl].T),
            "decT": np.ascontiguousarray(decoder_out[sl].T),
            "w2rep": w2rep,
            "b_attn": b_attn_2d,
            "b_comb": b_comb_2d,
        })
    return in_maps


def kernel(**inputs):
    from concourse.bass_utils import run_bass_kernel_spmd

    in_maps = make_in_maps(inputs)
    nc = _get_nc()
    res = run_bass_kernel_spmd(nc, in_maps, list(range(NCORES)))
    out = np.concatenate([res.results[c]["out"] for c in range(NCORES)], axis=0)
    applied = np.concatenate(
        [np.ascontiguousarray(res.results[c]["appliedT"].T)
         for c in range(NCORES)], axis=0)
    return out.astype(np.float32), applied.astype(np.float32)


# revision 6
# speedup vs baseline: 1.5317x; 1.1862x over previous
"""Trainium2 Bass kernel for nn_AttentionModule (Bahdanau-style attention).

Reference computation (S=512, B=64, H=1024, F=2H):
    cat    = concat([hidden bcast to (S,B,H), encoder_states], -1)      [S,B,2H]
    scores = tanh(cat @ W_attn.T + b_attn) @ W_attn2.T + b_attn2        [S,B,1]
    attn   = softmax(scores[..., 0].T, axis=-1)                         [B,S]
    applied= einsum("bs,sbh->bh", attn, encoder_states)                 [B,H]
    out    = tanh(concat([decoder_out, applied], -1) @ W_comb.T + b_comb)

Sharding: data-parallel over B across 8 cores (8 batch rows per core).

v2 (291.8us, from 376.7us bf16 baseline):
  - Main matmul in fp8e4 with perf_mode=DoubleRow (paired-K 3D APs,
    K=256/instruction).  W_attn pre-scaled by 8192 on the host so all
    values are fp8-normal; un-scaled via the tanh activation's scale.
    Encoder ships twice: fp8 (matmul) + bf16 (applied einsum).
  - applied einsum: fused mult+free-axis-accumulate per chunk via DVE
    scalar_tensor_tensor (accum_out).  NB tensor_tensor_reduce
    hard-faults the DVE on HW; STT is the working fusion.

v3: attacks the ~75us DMA-bound startup and the tail seen in the v2
trace (PE was 100% busy 80..260us but idle before/after):
  - All small fp32 constants (hidT/decT/w2rep) packed into ONE [128,256]
    blob DMA; v2 issued 32 scattered DMAs with 32-byte partition rows
    which serialized the startup queues for tens of us.
  - W1 (preamble weight, 4MB bf16) resident in SBUF, 8 big DMAs issued
    before everything else since the in-order PE must run the preamble
    first.
  - Score matmuls interleaved into the ft loop (emitted after the next
    ft's DR group) -> denser PE stream, tanh pool 18 -> 6 tiles.
  - Final-combine decoder half + wct loads emitted mid main-loop;
    only the applied half + bias remain after the last batch row.
  - appliedT output written SBUF-layout [128, KH*BL] in one DMA and
    unpacked on the host (v2 wrote 8 DMAs with 32-byte rows).

Known pitfalls baked in:
  - bf16/f32 input arrays with tiny (<=32B) partition rows are slow and
    (bf16) corruption-prone on the host->device path: pack/ship big.
  - fp8 without DoubleRow runs at bf16 speed.
  - 16/32-bit matmul operand mixing is rejected by the compiler.
  - mybir float8e4 == ml_dtypes.float8_e4m3 (IEEE, max normal 240).
  - tensor_tensor_reduce and gpsimd scalar_tensor_tensor are broken
    (HW fault / compile reject); DVE scalar_tensor_tensor works.
"""

import numpy as np

S, B, H = 512, 64, 1024
F = 2 * H
NCORES = 8
BL = B // NCORES          # 8 batch rows per core
KH = H // 128             # 8 contraction chunks over H
KF = F // 128             # 16 feature tiles
WSCALE = 8192.0           # host pre-scale on W_attn enc-half (fp8 range)

_CACHE = {}


def _build(num_devices=NCORES):
    from contextlib import ExitStack

    import concourse.tile as tile
    from concourse import bacc, mybir
    from concourse.masks import make_identity

    f32 = mybir.dt.float32
    bf16 = mybir.dt.bfloat16
    fp8 = mybir.dt.float8e4
    AF = mybir.ActivationFunctionType
    ALU = mybir.AluOpType
    AX = mybir.AxisListType
    DR = mybir.MatmulPerfMode.DoubleRow

    nc = bacc.Bacc("TRN2", target_bir_lowering=False, debug=False,
                   num_devices=num_devices)

    enc8_d = nc.dram_tensor("enc8", [BL, 128, KH * S], fp8,
                            kind="ExternalInput").ap()
    encb_d = nc.dram_tensor("encb", [BL, 128, KH * S], bf16,
                            kind="ExternalInput").ap()
    wat8 = nc.dram_tensor("wat8", [H, F], fp8, kind="ExternalInput").ap()
    w1t = nc.dram_tensor("w1t", [H, F], bf16, kind="ExternalInput").ap()
    wct = nc.dram_tensor("wct", [F, H], bf16, kind="ExternalInput").ap()
    blob_d = nc.dram_tensor("blob", [128, 2 * KH * BL + KF * BL], f32,
                            kind="ExternalInput").ap()
    b_attn_d = nc.dram_tensor("b_attn", [1, F], f32, kind="ExternalInput").ap()
    b_comb_d = nc.dram_tensor("b_comb", [1, H], f32, kind="ExternalInput").ap()
    out_d = nc.dram_tensor("out", [BL, H], f32, kind="ExternalOutput").ap()
    appT_d = nc.dram_tensor("appliedT", [128, KH * BL], f32,
                            kind="ExternalOutput").ap()

    with tile.TileContext(nc) as tc:
        with ExitStack() as ctx:
            consts = ctx.enter_context(tc.tile_pool(name="consts", bufs=1))
            enc8_pool = ctx.enter_context(tc.tile_pool(name="enc8", bufs=2))
            encb_pool = ctx.enter_context(tc.tile_pool(name="encb", bufs=2))
            tanh_pool = ctx.enter_context(tc.tile_pool(name="tanh", bufs=6))
            attn_pool = ctx.enter_context(tc.tile_pool(name="attn", bufs=2))
            abc_pool = ctx.enter_context(tc.tile_pool(name="abc", bufs=2))
            dram_pool = ctx.enter_context(
                tc.tile_pool(name="dram", bufs=2, space="DRAM"))
            scr_pool = ctx.enter_context(tc.tile_pool(name="scr", bufs=2))
            small_pool = ctx.enter_context(tc.tile_pool(name="small", bufs=4))
            wct_pool = ctx.enter_context(tc.tile_pool(name="wct", bufs=4))
            psT_pool = ctx.enter_context(
                tc.tile_pool(name="psT", bufs=2, space="PSUM"))
            psSc_pool = ctx.enter_context(
                tc.tile_pool(name="psSc", bufs=2, space="PSUM"))
            psPre_pool = ctx.enter_context(
                tc.tile_pool(name="psPre", bufs=2, space="PSUM"))
            psOut_pool = ctx.enter_context(
                tc.tile_pool(name="psOut", bufs=2, space="PSUM"))

            # ---- W1 resident, first in the DMA queues: the in-order PE
            # runs the preamble first, so these bytes gate everything ----
            w1_sb = consts.tile([128, KH * F], bf16)
            for kc in range(KH):
                nc.sync.dma_start(w1_sb[:, kc * F:(kc + 1) * F],
                                  w1t[kc * 128:(kc + 1) * 128, :])

            # ---- packed small constants: one DMA ----
            NB = 2 * KH * BL + KF * BL
            blob_sb = consts.tile([128, NB], f32)
            nc.sync.dma_start(blob_sb[:], blob_d[:])
            b_attn_32 = consts.tile([1, F], f32)
            nc.sync.dma_start(b_attn_32[:], b_attn_d[:])
            b_comb_32 = consts.tile([1, H], f32)
            nc.sync.dma_start(b_comb_32[:], b_comb_d[:])

            # ---- fp8 W2T chunks (2MB) ----
            w2t_sb = consts.tile([128, KH * F], fp8)
            for kc in range(KH):
                nc.sync.dma_start(
                    w2t_sb[:, kc * F:(kc + 1) * F],
                    wat8[kc * 128:(kc + 1) * 128, :])
            w2t3 = w2t_sb.rearrange("p (k f) -> p k f", f=F)

            # ---- encoder prefetch (fp8 for matmul, bf16 for applied) ----
            def load_enc8(b):
                t = enc8_pool.tile([128, KH * S], fp8, tag="enc8", name="enc8")
                nc.sync.dma_start(t[:], enc8_d[b])
                return t

            def load_encb(b):
                t = encb_pool.tile([128, KH * S], bf16, tag="encb",
                                   name="encb")
                nc.sync.dma_start(t[:], encb_d[b])
                return t

            enc8_tiles = {0: load_enc8(0)}
            encb_tiles = {0: load_encb(0)}

            # ---- device-side constant prep ----
            identity = consts.tile([128, 128], f32)
            make_identity(nc, identity[:])
            ones_bf = consts.tile([1, BL], bf16)
            nc.vector.memset(ones_bf[:], 1.0)
            b_attn_sb = consts.tile([1, F], bf16)
            nc.vector.tensor_copy(b_attn_sb[:], b_attn_32[:])
            b_comb_sb = consts.tile([1, H], bf16)
            nc.vector.tensor_copy(b_comb_sb[:], b_comb_32[:])
            hidT_sb = consts.tile([128, KH * BL], bf16)
            nc.vector.tensor_copy(hidT_sb[:], blob_sb[:, 0:KH * BL])
            decT_sb = consts.tile([128, KH * BL], bf16)
            nc.vector.tensor_copy(decT_sb[:],
                                  blob_sb[:, KH * BL:2 * KH * BL])
            w2rep_sb = consts.tile([128, KF * BL], bf16)
            nc.vector.tensor_copy(w2rep_sb[:], blob_sb[:, 2 * KH * BL:NB])

            hidbT_sb = consts.tile([128, KF * BL], f32)
            appT_sb = consts.tile([128, KH * BL], f32)
            appT_bf = consts.tile([128, KH * BL], bf16)

            # ---- hid_part preamble: hidb[b, f] = hidden @ W1.T + b_attn ----
            hidb_row = consts.tile([BL, F], f32)
            for fc in range(F // 512):
                ph = psPre_pool.tile([BL, 512], f32, tag="pre", name=f"ph{fc}")
                for kc in range(KH):
                    nc.tensor.matmul(
                        ph[:], hidT_sb[:, kc * BL:(kc + 1) * BL],
                        w1_sb[:, kc * F + fc * 512: kc * F + (fc + 1) * 512],
                        start=(kc == 0), stop=False)
                nc.tensor.matmul(
                    ph[:], ones_bf[:], b_attn_sb[:, fc * 512:(fc + 1) * 512],
                    start=False, stop=True)
                nc.vector.tensor_copy(hidb_row[:, fc * 512:(fc + 1) * 512],
                                      ph[:])
            # transpose [8, 2048] -> hidbT_sb [128, KF*8] (f on partitions)
            for ft in range(KF):
                ptp = psPre_pool.tile([128, BL], f32, tag="pre", name="ptp")
                nc.tensor.transpose(ptp[:],
                                    hidb_row[:, ft * 128:(ft + 1) * 128],
                                    identity[:BL, :BL])
                nc.vector.tensor_copy(hidbT_sb[:, ft * BL:(ft + 1) * BL],
                                      ptp[:])

            # final-combine psum, accumulated across the whole main loop
            pouts = [psOut_pool.tile([BL, 512], f32, tag="pout", name=f"po{i}")
                     for i in range(2)]

            def emit_dec_half():
                for kc in range(KH):
                    w = wct_pool.tile([128, H], bf16, tag="wct", name="wctt")
                    nc.sync.dma_start(w[:], wct[kc * 128:(kc + 1) * 128, :])
                    lhs = decT_sb[:, kc * BL:(kc + 1) * BL]
                    for fc in range(2):
                        nc.tensor.matmul(
                            pouts[fc][:], lhs, w[:, fc * 512:(fc + 1) * 512],
                            start=(kc == 0), stop=False,
                            skip_group_check=True)

            # ---- main loop over local batch rows ----
            for b in range(BL):
                if b + 1 < BL:
                    enc8_tiles[b + 1] = load_enc8(b + 1)
                    encb_tiles[b + 1] = load_encb(b + 1)
                if b == 2:
                    emit_dec_half()
                e8 = enc8_tiles.pop(b)
                eb = encb_tiles.pop(b)
                e83 = e8.rearrange("p (k s) -> p k s", s=S)

                def ebk(kc):
                    return eb[:, kc * S:(kc + 1) * S]

                psc = psSc_pool.tile([BL, S], f32, tag="psc", name="psc")
                th = {}
                for ft in range(KF):
                    pT = psT_pool.tile([128, S], f32, tag="pT", name="pT")
                    for kc2 in range(KH // 2):
                        nc.tensor.matmul(
                            pT[:],
                            w2t3[:, 2 * kc2:2 * kc2 + 2,
                                 ft * 128:(ft + 1) * 128],
                            e83[:, 2 * kc2:2 * kc2 + 2, :],
                            start=(kc2 == 0), stop=(kc2 == KH // 2 - 1),
                            perf_mode=DR, skip_group_check=True)
                    t = tanh_pool.tile([128, S], bf16, tag="tanh", name="tanh")
                    nc.scalar.activation(
                        t[:], pT[:], AF.Tanh,
                        bias=hidbT_sb[:, ft * BL + b: ft * BL + b + 1],
                        scale=1.0 / WSCALE)
                    th[ft] = t
                    # interleave score matmul for the PREVIOUS ft: its tanh
                    # has drained by now, keeping the PE stream dense while
                    # letting th tiles retire quickly.
                    if ft > 0:
                        nc.tensor.matmul(
                            psc[:], w2rep_sb[:, (ft - 1) * BL:ft * BL],
                            th.pop(ft - 1)[:],
                            start=(ft - 1 == 0), stop=False,
                            skip_group_check=True)
                nc.tensor.matmul(
                    psc[:], w2rep_sb[:, (KF - 1) * BL:KF * BL],
                    th.pop(KF - 1)[:],
                    start=False, stop=True, skip_group_check=True)

                negmax = small_pool.tile([BL, 1], f32, tag="negmax",
                                         name="negmax")
                nc.vector.reduce_max(negmax[:], psc[:], axis=AX.X, negate=True)
                attn = attn_pool.tile([BL, S], bf16, tag="attn", name="attn")
                sumexp = small_pool.tile([BL, 1], f32, tag="sumexp",
                                         name="sumexp")
                nc.scalar.activation(attn[:], psc[:], AF.Exp,
                                     bias=negmax[:], scale=1.0,
                                     accum_out=sumexp[:])
                recip = small_pool.tile([BL, 1], f32, tag="recip", name="recip")
                nc.vector.reciprocal(recip[:], sumexp[:])
                nc.vector.tensor_scalar_mul(attn[:], attn[:], recip[:])

                # broadcast attn row across 128 partitions via DRAM bounce
                attn_dr = dram_pool.tile([1, S], bf16, tag="attn_dr",
                                         name="attn_dr")
                nc.sync.dma_start(attn_dr[:], attn[0:1, :])
                abc = abc_pool.tile([128, S], bf16, tag="abc", name="abc")
                nc.sync.dma_start(abc[:],
                                  attn_dr[0:1, :].to_broadcast((128, S)))

                # applied^T column b: fused mult+accum per chunk on VectorE
                for kc in range(KH):
                    col = appT_sb[:, kc * BL + b: kc * BL + b + 1]
                    scr = scr_pool.tile([128, S], bf16, tag="scr",
                                        name="scr")
                    nc.vector.scalar_tensor_tensor(
                        out=scr[:], in0=ebk(kc), scalar=1.0, in1=abc[:],
                        op0=ALU.mult, op1=ALU.mult, accum_out=col)
                nc.vector.tensor_copy(
                    appT_bf.rearrange("p (k b) -> p k b", b=BL)[:, :, b],
                    appT_sb.rearrange("p (k b) -> p k b", b=BL)[:, :, b])

            # ---- final combine: applied half + bias ----
            for kc in range(KH):
                w = wct_pool.tile([128, H], bf16, tag="wct", name="wctt")
                nc.sync.dma_start(w[:], wct[(KH + kc) * 128:
                                            (KH + kc + 1) * 128, :])
                lhs = appT_bf[:, kc * BL:(kc + 1) * BL]
                for fc in range(2):
                    nc.tensor.matmul(
                        pouts[fc][:], lhs, w[:, fc * 512:(fc + 1) * 512],
                        start=False, stop=False, skip_group_check=True)
            for fc in range(2):
                nc.tensor.matmul(
                    pouts[fc][:], ones_bf[:],
                    b_comb_sb[:, fc * 512:(fc + 1) * 512],
                    start=False, stop=True, skip_group_check=True)

            out_sb = consts.tile([BL, H], f32)
            for fc in range(2):
                nc.scalar.activation(out_sb[:, fc * 512:(fc + 1) * 512],
                                     pouts[fc][:], AF.Tanh)
            nc.sync.dma_start(out_d[:], out_sb[:])
            nc.sync.dma_start(appT_d[:], appT_sb[:])

    nc.compile()
    return nc


def _get_nc():
    if "nc" not in _CACHE:
        _CACHE["nc"] = _build()
    return _CACHE["nc"]


def make_in_maps(inputs):
    import ml_dtypes
    bf = ml_dtypes.bfloat16
    f8 = ml_dtypes.float8_e4m3

    inp = {k: np.asarray(v, dtype=np.float32) for k, v in inputs.items()}
    hidden = inp["hidden"]
    decoder_out = inp["decoder_out"]
    encoder_states = inp["encoder_states"]
    W_attn = inp["W_attn"]
    b_attn = inp["b_attn"]
    W_attn2 = inp["W_attn2"]
    W_comb = inp["W_comb"]
    b_comb = inp["b_comb"]
    # b_attn2 shifts every score equally -> softmax-invariant, unused.

    watT = np.ascontiguousarray(W_attn.T)            # [2H, F]
    wat8 = np.ascontiguousarray(watT[H:] * WSCALE).astype(f8)
    w1t_np = np.ascontiguousarray(watT[:H]).astype(bf)
    wct = np.ascontiguousarray(W_comb.T).astype(bf)
    b_attn_2d = np.ascontiguousarray(b_attn.reshape(1, F))
    b_comb_2d = np.ascontiguousarray(b_comb.reshape(1, H))
    # w2rep block of the blob: [128, KF*BL], col ft*BL+b = W2[ft*128+p]
    w2cols = W_attn2.reshape(KF, 128).T              # [128, KF]
    w2rep_blk = np.repeat(w2cols[:, :, None], BL, axis=2).reshape(128, KF * BL)

    def pack_T(x):  # [BLx, H] -> [128, KH*BLx] with col kc*BLx+b
        blx = x.shape[0]
        return np.ascontiguousarray(
            x.T.reshape(KH, 128, blx).transpose(1, 0, 2).reshape(
                128, KH * blx))

    in_maps = []
    for c in range(NCORES):
        sl = slice(c * BL, (c + 1) * BL)
        # [S, BL, H] -> [BL, H, S] -> [BL, 128, KH, S] (partition-major)
        encT = encoder_states[:, sl, :].transpose(1, 2, 0)
        encP = np.ascontiguousarray(
            encT.reshape(BL, KH, 128, S).transpose(0, 2, 1, 3)
        ).reshape(BL, 128, KH * S)
        blob = np.concatenate(
            [pack_T(hidden[sl]), pack_T(decoder_out[sl]), w2rep_blk],
            axis=1).astype(np.float32)
        in_maps.append({
            "enc8": encP.astype(f8),
            "encb": encP.astype(bf),
            "wat8": wat8,
            "w1t": w1t_np,
            "wct": wct,
            "blob": np.ascontiguousarray(blob),
            "b_attn": b_attn_2d,
            "b_comb": b_comb_2d,
        })
    return in_maps


def kernel(**inputs):
    from concourse.bass_utils import run_bass_kernel_spmd

    in_maps = make_in_maps(inputs)
    nc = _get_nc()
    res = run_bass_kernel_spmd(nc, in_maps, list(range(NCORES)))
    out = np.concatenate([res.results[c]["out"] for c in range(NCORES)], axis=0)
    applied_parts = []
    for c in range(NCORES):
        arr = res.results[c]["appliedT"]          # [128, KH*BL]
        applied_parts.append(
            arr.reshape(128, KH, BL).transpose(2, 1, 0).reshape(BL, H))
    applied = np.concatenate(applied_parts, axis=0)
    return out.astype(np.float32), applied.astype(np.float32)


# revision 13
# speedup vs baseline: 1.6359x; 1.0680x over previous
"""Trainium2 Bass kernel for nn_AttentionModule (Bahdanau-style attention).

Reference computation (S=512, B=64, H=1024, F=2H):
    cat    = concat([hidden bcast to (S,B,H), encoder_states], -1)      [S,B,2H]
    scores = tanh(cat @ W_attn.T + b_attn) @ W_attn2.T + b_attn2        [S,B,1]
    attn   = softmax(scores[..., 0].T, axis=-1)                         [B,S]
    applied= einsum("bs,sbh->bh", attn, encoder_states)                 [B,H]
    out    = tanh(concat([decoder_out, applied], -1) @ W_comb.T + b_comb)

Sharding: data-parallel over B across 8 cores (8 batch rows per core).

v2 (291.8us, from 376.7us bf16 baseline):
  - Main matmul in fp8e4 with perf_mode=DoubleRow (paired-K 3D APs,
    K=256/instruction).  W_attn pre-scaled by 8192 on the host so all
    values are fp8-normal; un-scaled via the tanh activation's scale.
    Encoder ships twice: fp8 (matmul) + bf16 (applied einsum).
  - applied einsum: fused mult+free-axis-accumulate per chunk via DVE
    scalar_tensor_tensor (accum_out).  NB tensor_tensor_reduce
    hard-faults the DVE on HW; STT is the working fusion.

v3: attacks the ~75us DMA-bound startup and the tail seen in the v2
trace (PE was 100% busy 80..260us but idle before/after):
  - All small fp32 constants (hidT/decT/w2rep) packed into ONE [128,256]
    blob DMA; v2 issued 32 scattered DMAs with 32-byte partition rows
    which serialized the startup queues for tens of us.
  - W1 (preamble weight, 4MB bf16) resident in SBUF, 8 big DMAs issued
    before everything else since the in-order PE must run the preamble
    first.
  - Score matmuls interleaved into the ft loop (emitted after the next
    ft's DR group) -> denser PE stream, tanh pool 18 -> 6 tiles.
  - Final-combine decoder half + wct loads emitted mid main-loop;
    only the applied half + bias remain after the last batch row.
  - appliedT output written SBUF-layout [128, KH*BL] in one DMA and
    unpacked on the host (v2 wrote 8 DMAs with 32-byte rows).

Known pitfalls baked in:
  - bf16/f32 input arrays with tiny (<=32B) partition rows are slow and
    (bf16) corruption-prone on the host->device path: pack/ship big.
  - fp8 without DoubleRow runs at bf16 speed.
  - 16/32-bit matmul operand mixing is rejected by the compiler.
  - mybir float8e4 == ml_dtypes.float8_e4m3 (IEEE, max normal 240).
  - tensor_tensor_reduce and gpsimd scalar_tensor_tensor are broken
    (HW fault / compile reject); DVE scalar_tensor_tensor works.
"""

import numpy as np

S, B, H = 512, 64, 1024
F = 2 * H
NCORES = 8
BL = B // NCORES          # 8 batch rows per core
KH = H // 128             # 8 contraction chunks over H
KF = F // 128             # 16 feature tiles
WSCALE = 8192.0           # host pre-scale on W_attn enc-half (fp8 range)

_CACHE = {}


def _build(num_devices=NCORES):
    from contextlib import ExitStack

    import concourse.tile as tile
    from concourse import bacc, mybir
    from concourse.masks import make_identity

    f32 = mybir.dt.float32
    bf16 = mybir.dt.bfloat16
    fp8 = mybir.dt.float8e4
    AF = mybir.ActivationFunctionType
    ALU = mybir.AluOpType
    AX = mybir.AxisListType
    DR = mybir.MatmulPerfMode.DoubleRow

    nc = bacc.Bacc("TRN2", target_bir_lowering=False, debug=False,
                   num_devices=num_devices)

    enc8_d = nc.dram_tensor("enc8", [BL, 128, KH * S], fp8,
                            kind="ExternalInput").ap()
    encb_d = nc.dram_tensor("encb", [BL, 128, KH * S], bf16,
                            kind="ExternalInput").ap()
    wat8 = nc.dram_tensor("wat8", [H, F], fp8, kind="ExternalInput").ap()
    w1t = nc.dram_tensor("w1t", [H, F], bf16, kind="ExternalInput").ap()
    wct = nc.dram_tensor("wct", [F, H], bf16, kind="ExternalInput").ap()
    blob_d = nc.dram_tensor("blob", [128, 2 * KH * BL + KF * BL], f32,
                            kind="ExternalInput").ap()
    b_attn_d = nc.dram_tensor("b_attn", [1, F], f32, kind="ExternalInput").ap()
    b_comb_d = nc.dram_tensor("b_comb", [1, H], f32, kind="ExternalInput").ap()
    out_d = nc.dram_tensor("out", [BL, H], f32, kind="ExternalOutput").ap()
    appT_d = nc.dram_tensor("appliedT", [128, KH * BL], f32,
                            kind="ExternalOutput").ap()

    with tile.TileContext(nc) as tc:
        with ExitStack() as ctx:
            consts = ctx.enter_context(tc.tile_pool(name="consts", bufs=1))
            enc8_pool = ctx.enter_context(tc.tile_pool(name="enc8", bufs=4))
            encb_pool = ctx.enter_context(tc.tile_pool(name="encb", bufs=4))
            tanh_pool = ctx.enter_context(tc.tile_pool(name="tanh", bufs=6))
            attn_pool = ctx.enter_context(tc.tile_pool(name="attn", bufs=2))
            abc_pool = ctx.enter_context(tc.tile_pool(name="abc", bufs=2))
            dram_pool = ctx.enter_context(
                tc.tile_pool(name="dram", bufs=2, space="DRAM"))
            scr_pool = ctx.enter_context(tc.tile_pool(name="scr", bufs=2))
            small_pool = ctx.enter_context(tc.tile_pool(name="small", bufs=4))
            wct_pool = ctx.enter_context(tc.tile_pool(name="wct", bufs=4))
            wct2_pool = ctx.enter_context(tc.tile_pool(name="wct2", bufs=8))
            psT_pool = ctx.enter_context(
                tc.tile_pool(name="psT", bufs=4, space="PSUM"))
            psSc_pool = ctx.enter_context(
                tc.tile_pool(name="psSc", bufs=2, space="PSUM"))
            psOut_pool = ctx.enter_context(
                tc.tile_pool(name="psOut", bufs=2, space="PSUM"))

            # ---- W1 resident, first in the DMA queues: the in-order PE
            # runs the preamble first, so these bytes gate everything ----
            w1_sb = consts.tile([128, KH * F], bf16)
            for kc in range(KH):
                nc.sync.dma_start(w1_sb[:, kc * F:(kc + 1) * F],
                                  w1t[kc * 128:(kc + 1) * 128, :])

            # ---- packed small constants: one DMA ----
            NB = 2 * KH * BL + KF * BL
            blob_sb = consts.tile([128, NB], f32)
            nc.sync.dma_start(blob_sb[:], blob_d[:])
            b_attn_32 = consts.tile([1, F], f32)
            nc.sync.dma_start(b_attn_32[:], b_attn_d[:])
            b_comb_32 = consts.tile([1, H], f32)
            nc.sync.dma_start(b_comb_32[:], b_comb_d[:])

            # ---- fp8 W2T chunks (2MB) ----
            w2t_sb = consts.tile([128, KH * F], fp8)
            for kc in range(KH):
                nc.sync.dma_start(
                    w2t_sb[:, kc * F:(kc + 1) * F],
                    wat8[kc * 128:(kc + 1) * 128, :])
            w2t3 = w2t_sb.rearrange("p (k f) -> p k f", f=F)

            # ---- encoder prefetch (fp8 for matmul, bf16 for applied) ----
            def load_enc8(b):
                t = enc8_pool.tile([128, KH * S], fp8, tag="enc8", name="enc8")
                nc.sync.dma_start(t[:], enc8_d[b])
                return t

            def load_encb(b):
                t = encb_pool.tile([128, KH * S], bf16, tag="encb",
                                   name="encb")
                nc.sync.dma_start(t[:], encb_d[b])
                return t

            enc8_tiles = {0: load_enc8(0), 1: load_enc8(1)}
            encb_tiles = {0: load_encb(0), 1: load_encb(1)}

            # ---- device-side constant prep ----
            identity = consts.tile([128, 128], f32)
            make_identity(nc, identity[:])
            ones_bf = consts.tile([1, BL], bf16)
            nc.vector.memset(ones_bf[:], 1.0)
            b_attn_sb = consts.tile([1, F], bf16)
            nc.vector.tensor_copy(b_attn_sb[:], b_attn_32[:])
            b_comb_sb = consts.tile([1, H], bf16)
            nc.vector.tensor_copy(b_comb_sb[:], b_comb_32[:])
            hidT_sb = consts.tile([128, KH * BL], bf16)
            nc.vector.tensor_copy(hidT_sb[:], blob_sb[:, 0:KH * BL])
            decT_sb = consts.tile([128, KH * BL], bf16)
            nc.vector.tensor_copy(decT_sb[:],
                                  blob_sb[:, KH * BL:2 * KH * BL])
            w2rep_sb = consts.tile([128, KF * BL], bf16)
            nc.vector.tensor_copy(w2rep_sb[:], blob_sb[:, 2 * KH * BL:NB])

            hidbT_sb = consts.tile([128, KF * BL], f32)
            appT_sb = consts.tile([128, KH * BL], f32)
            appT_bf = consts.tile([128, KH * BL], bf16)

            # ---- hid_part preamble: hidb[b, f] = hidden @ W1.T + b_attn ----
            hidb_row = consts.tile([BL, F], f32)
            for fc in range(F // 512):
                ph = psT_pool.tile([BL, 512], f32, tag="pT", name=f"ph{fc}")
                for kc in range(KH):
                    nc.tensor.matmul(
                        ph[:], hidT_sb[:, kc * BL:(kc + 1) * BL],
                        w1_sb[:, kc * F + fc * 512: kc * F + (fc + 1) * 512],
                        start=(kc == 0), stop=False)
                nc.tensor.matmul(
                    ph[:], ones_bf[:], b_attn_sb[:, fc * 512:(fc + 1) * 512],
                    start=False, stop=True)
                nc.vector.tensor_copy(hidb_row[:, fc * 512:(fc + 1) * 512],
                                      ph[:])
            # transpose [8, 2048] -> hidbT_sb [128, KF*8] (f on partitions)
            for ft in range(KF):
                ptp = psT_pool.tile([128, BL], f32, tag="pT", name="ptp")
                nc.tensor.transpose(ptp[:],
                                    hidb_row[:, ft * 128:(ft + 1) * 128],
                                    identity[:BL, :BL])
                nc.vector.tensor_copy(hidbT_sb[:, ft * BL:(ft + 1) * BL],
                                      ptp[:])

            # final-combine psum: allocated at b-pair 1 (after the preamble
            # has released the borrowed psOut banks), accumulated to the end
            pouts = []
            wct2 = []

            def emit_dec_half():
                pouts.extend(
                    psOut_pool.tile([BL, 512], f32, tag="pout", name=f"po{i}")
                    for i in range(2))
                for kc in range(KH):
                    w = wct_pool.tile([128, H], bf16, tag="wct", name="wctt")
                    nc.sync.dma_start(w[:], wct[kc * 128:(kc + 1) * 128, :])
                    lhs = decT_sb[:, kc * BL:(kc + 1) * BL]
                    for fc in range(2):
                        nc.tensor.matmul(
                            pouts[fc][:], lhs, w[:, fc * 512:(fc + 1) * 512],
                            start=(kc == 0), stop=False,
                            skip_group_check=True)
                # prefetch the applied-half weights; resident until the end
                for kc in range(KH):
                    w = wct2_pool.tile([128, H], bf16, tag="wct2",
                                       name="wct2")
                    nc.sync.dma_start(w[:], wct[(KH + kc) * 128:
                                                (KH + kc + 1) * 128, :])
                    wct2.append(w)

            # ---- main loop over pairs of local batch rows: the stationary
            # DR weight chunk is loaded once per (ft, kc2) and reused by both
            # rows, halving LDWEIGHTS pressure ----
            for g in range(BL // 2):
                bs = [2 * g, 2 * g + 1]
                for b in (2 * g + 2, 2 * g + 3):
                    if b < BL:
                        enc8_tiles[b] = load_enc8(b)
                        encb_tiles[b] = load_encb(b)
                if g == 1:
                    emit_dec_half()
                e83s = {}
                ebs = {}
                for b in bs:
                    e83s[b] = enc8_tiles.pop(b).rearrange(
                        "p (k s) -> p k s", s=S)
                    ebs[b] = encb_tiles.pop(b)

                pscs = {b: psSc_pool.tile([BL, S], f32, tag="psc",
                                          name="psc") for b in bs}
                th = {}
                for ft in range(KF):
                    pTs = {b: psT_pool.tile([128, S], f32, tag="pT",
                                            name="pT") for b in bs}
                    for kc2 in range(KH // 2):
                        lhsT = w2t3[:, 2 * kc2:2 * kc2 + 2,
                                    ft * 128:(ft + 1) * 128]
                        for b in bs:
                            nc.tensor.matmul(
                                pTs[b][:], lhsT,
                                e83s[b][:, 2 * kc2:2 * kc2 + 2, :],
                                start=(kc2 == 0), stop=(kc2 == KH // 2 - 1),
                                perf_mode=DR, skip_group_check=True)
                    for b in bs:
                        t = tanh_pool.tile([128, S], bf16, tag="tanh",
                                           name="tanh")
                        nc.scalar.activation(
                            t[:], pTs[b][:], AF.Tanh,
                            bias=hidbT_sb[:, ft * BL + b: ft * BL + b + 1],
                            scale=1.0 / WSCALE)
                        th[b, ft] = t
                    # interleave the previous ft's score matmuls
                    if ft > 0:
                        for b in bs:
                            nc.tensor.matmul(
                                pscs[b][:],
                                w2rep_sb[:, (ft - 1) * BL:ft * BL],
                                th.pop((b, ft - 1))[:],
                                start=(ft - 1 == 0), stop=False,
                                skip_group_check=True)
                for b in bs:
                    nc.tensor.matmul(
                        pscs[b][:], w2rep_sb[:, (KF - 1) * BL:KF * BL],
                        th.pop((b, KF - 1))[:],
                        start=False, stop=True, skip_group_check=True)

                for b in bs:
                    psc = pscs[b]
                    eb = ebs[b]
                    negmax = small_pool.tile([BL, 1], f32, tag="negmax",
                                             name="negmax")
                    nc.vector.reduce_max(negmax[:], psc[:], axis=AX.X,
                                         negate=True)
                    attn = attn_pool.tile([BL, S], bf16, tag="attn",
                                          name="attn")
                    sumexp = small_pool.tile([BL, 1], f32, tag="sumexp",
                                             name="sumexp")
                    nc.scalar.activation(attn[:], psc[:], AF.Exp,
                                         bias=negmax[:], scale=1.0,
                                         accum_out=sumexp[:])
                    recip = small_pool.tile([BL, 1], f32, tag="recip",
                                            name="recip")
                    nc.vector.reciprocal(recip[:], sumexp[:])
                    nc.vector.tensor_scalar_mul(attn[:], attn[:], recip[:])

                    # broadcast attn row across 128 partitions (DRAM bounce)
                    attn_dr = dram_pool.tile([1, S], bf16, tag="attn_dr",
                                             name="attn_dr")
                    nc.sync.dma_start(attn_dr[:], attn[0:1, :])
                    abc = abc_pool.tile([128, S], bf16, tag="abc", name="abc")
                    nc.sync.dma_start(abc[:],
                                      attn_dr[0:1, :].to_broadcast((128, S)))

                    # applied^T column b: fused mult+accum per chunk (DVE)
                    for kc in range(KH):
                        col = appT_sb[:, kc * BL + b: kc * BL + b + 1]
                        scr = scr_pool.tile([128, S], bf16, tag="scr",
                                            name="scr")
                        nc.vector.scalar_tensor_tensor(
                            out=scr[:], in0=eb[:, kc * S:(kc + 1) * S],
                            scalar=1.0, in1=abc[:],
                            op0=ALU.mult, op1=ALU.mult, accum_out=col)
                    nc.vector.tensor_copy(
                        appT_bf.rearrange("p (k b) -> p k b", b=BL)[:, :, b],
                        appT_sb.rearrange("p (k b) -> p k b", b=BL)[:, :, b])

            # ---- final combine: applied half + bias ----
            for kc in range(KH):
                lhs = appT_bf[:, kc * BL:(kc + 1) * BL]
                for fc in range(2):
                    nc.tensor.matmul(
                        pouts[fc][:], lhs,
                        wct2[kc][:, fc * 512:(fc + 1) * 512],
                        start=False, stop=False, skip_group_check=True)
            for fc in range(2):
                nc.tensor.matmul(
                    pouts[fc][:], ones_bf[:],
                    b_comb_sb[:, fc * 512:(fc + 1) * 512],
                    start=False, stop=True, skip_group_check=True)

            out_sb = consts.tile([BL, H], f32)
            for fc in range(2):
                nc.scalar.activation(out_sb[:, fc * 512:(fc + 1) * 512],
                                     pouts[fc][:], AF.Tanh)
            nc.sync.dma_start(out_d[:], out_sb[:])
            nc.sync.dma_start(appT_d[:], appT_sb[:])

    nc.compile()
    return nc


def _get_nc():
    if "nc" not in _CACHE:
        _CACHE["nc"] = _build()
    return _CACHE["nc"]


def make_in_maps(inputs):
    import ml_dtypes
    bf = ml_dtypes.bfloat16
    f8 = ml_dtypes.float8_e4m3

    inp = {k: np.asarray(v, dtype=np.float32) for k, v in inputs.items()}
    hidden = inp["hidden"]
    decoder_out = inp["decoder_out"]
    encoder_states = inp["encoder_states"]
    W_attn = inp["W_attn"]
    b_attn = inp["b_attn"]
    W_attn2 = inp["W_attn2"]
    W_comb = inp["W_comb"]
    b_comb = inp["b_comb"]
    # b_attn2 shifts every score equally -> softmax-invariant, unused.

    watT = np.ascontiguousarray(W_attn.T)            # [2H, F]
    wat8 = np.ascontiguousarray(watT[H:] * WSCALE).astype(f8)
    w1t_np = np.ascontiguousarray(watT[:H]).astype(bf)
    wct = np.ascontiguousarray(W_comb.T).astype(bf)
    b_attn_2d = np.ascontiguousarray(b_attn.reshape(1, F))
    b_comb_2d = np.ascontiguousarray(b_comb.reshape(1, H))
    # w2rep block of the blob: [128, KF*BL], col ft*BL+b = W2[ft*128+p]
    w2cols = W_attn2.reshape(KF, 128).T              # [128, KF]
    w2rep_blk = np.repeat(w2cols[:, :, None], BL, axis=2).reshape(128, KF * BL)

    def pack_T(x):  # [BLx, H] -> [128, KH*BLx] with col kc*BLx+b
        blx = x.shape[0]
        return np.ascontiguousarray(
            x.T.reshape(KH, 128, blx).transpose(1, 0, 2).reshape(
                128, KH * blx))

    in_maps = []
    for c in range(NCORES):
        sl = slice(c * BL, (c + 1) * BL)
        # [S, BL, H] -> [BL, H, S] -> [BL, 128, KH, S] (partition-major)
        encT = encoder_states[:, sl, :].transpose(1, 2, 0)
        encP = np.ascontiguousarray(
            encT.reshape(BL, KH, 128, S).transpose(0, 2, 1, 3)
        ).reshape(BL, 128, KH * S)
        blob = np.concatenate(
            [pack_T(hidden[sl]), pack_T(decoder_out[sl]), w2rep_blk],
            axis=1).astype(np.float32)
        in_maps.append({
            "enc8": encP.astype(f8),
            "encb": encP.astype(bf),
            "wat8": wat8,
            "w1t": w1t_np,
            "wct": wct,
            "blob": np.ascontiguousarray(blob),
            "b_attn": b_attn_2d,
            "b_comb": b_comb_2d,
        })
    return in_maps


def kernel(**inputs):
    from concourse.bass_utils import run_bass_kernel_spmd

    in_maps = make_in_maps(inputs)
    nc = _get_nc()
    res = run_bass_kernel_spmd(nc, in_maps, list(range(NCORES)))
    out = np.concatenate([res.results[c]["out"] for c in range(NCORES)], axis=0)
    applied_parts = []
    for c in range(NCORES):
        arr = res.results[c]["appliedT"]          # [128, KH*BL]
        applied_parts.append(
            arr.reshape(128, KH, BL).transpose(2, 1, 0).reshape(BL, H))
    applied = np.concatenate(applied_parts, axis=0)
    return out.astype(np.float32), applied.astype(np.float32)
